# revision 55
# baseline (speedup 1.0000x reference)
"""Trainium2 Bass kernel for nn_Classical_autoencoder (patch MLP autoencoder + cosine fold).

Contract: kernel(**inputs) takes FULL inputs (img (32,1,512,512), W1 (16,4), b1 (4,),
W2 (4,4), b2 (4,), W3 (4,16), b3 (16,)) and returns the FULL (32,512,512) f32 output.
Internally: pure data-parallel over 8 NeuronCores, 4 images per core.

Math (per image):
  patches x = im2col(img, 4x4, stride 2)           # (255*255, 16)
  y = relu(relu(relu(x@W1+b1)@W2+b2)@W3+b3)        # (P, 16)
  S[i,j] = x.y / (max(|x|,eps)*max(|y|,eps))       # (255,255)
  out[r,c] = mean of S[i,j] for i in {r//2-1, r//2} & [0,255), j likewise
  (the overlapping fold with k=4,s=2 reduces exactly to this 2-tap box filter
   on S, upsampled 2x with 2x2-constant blocks)

V4 layout / engine plan (vs v3):
  img4c dropped: cols 2j+2+t are just rtb[..., j+1] (shift-by-one view), so
      one 512KB row tile per image instead of two.
  products: one 4-free-dim DVE op per ci (overlapping jo/j strides on the
      rtb read) instead of 4 ops -> 12 fewer DVE dispatches per image.
  cw4[32k+g, ci, 4g+ci] gather: ct matmul outputs land at partition q=4g+ci
      = patch rows (2q, 2q+1), i.e. already in fold row-pair order. The
      whole DRAM-bounce reorg of v3 (5 DMAs/image) disappears; all 4 ci
      accumulate into one full-width PSUM bank per quantity.
  single fused output DMA per image (row duplication via 0-stride read).
"""

import sys

for _p in ("/opt/trn_rl_repo", "/root/.axon_site/_ro/trn_rl_repo"):
    if _p not in sys.path:
        sys.path.append(_p)

from contextlib import ExitStack

import numpy as np

import concourse.bass as bass
import concourse.tile as tile
from concourse import bacc, mybir

F32 = mybir.dt.float32
BF16 = mybir.dt.bfloat16
F8 = mybir.dt.float8e4
ADD = mybir.AluOpType.add
MULT = mybir.AluOpType.mult
MAX = mybir.AluOpType.max

IMG = 512
OH = 255
NSAMP = 4
NCORES = 8

# GPSIMD cannot read PSUM (BIR verifier) -> all relus on ACT/DVE.
# ysq is SBUF-only, so GpSimd can take a share of it.
YSQ_GPS = (True, False, False, False)  # per ci
# products jo=1 op offloaded to GpSimd for these ci (DVE relief)
PROD_GPS = (True, True, True, True)


def build_nc() -> bass.Bass:
    nc = bacc.Bacc()

    img4b = nc.declare_dram_parameter("img4b", [NSAMP, 128, 8, 2, 256], BF16, isOutput=False)[:]
    l1w = nc.declare_dram_parameter("l1w", [128, 4, 128], BF16, isOutput=False)[:]
    l2w = nc.declare_dram_parameter("l2w", [128, 128], BF16, isOutput=False)[:]
    l3w = nc.declare_dram_parameter("l3w", [128, 4, 128], BF16, isOutput=False)[:]
    b3v = nc.declare_dram_parameter("b3v", [128, 4], F32, isOutput=False)[:]
    cw4 = nc.declare_dram_parameter("cw4", [128, 4, 128], BF16, isOutput=False)[:]
    fw = nc.declare_dram_parameter("fw", [128, 4, 128], BF16, isOutput=False)[:]
    b1v = nc.declare_dram_parameter("b1v", [128, 1], F32, isOutput=False)[:]
    b2v = nc.declare_dram_parameter("b2v", [128, 1], F32, isOutput=False)[:]
    out4 = nc.declare_dram_parameter("out4", [NSAMP, 2, 2, 128, IMG], BF16, isOutput=True)[:]

    with ExitStack() as ctx:
        tc = ctx.enter_context(tile.TileContext(nc))
        consts = ctx.enter_context(tc.tile_pool(name="consts", bufs=1))
        rows = ctx.enter_context(tc.tile_pool(name="rows", bufs=2))
        xsq = ctx.enter_context(tc.tile_pool(name="xsq", bufs=2))
        mlp = ctx.enter_context(tc.tile_pool(name="mlp", bufs=3))
        tailp = ctx.enter_context(tc.tile_pool(name="tailp", bufs=2))
        foldp = ctx.enter_context(tc.tile_pool(name="foldp", bufs=2))
        psz = ctx.enter_context(tc.tile_pool(name="psz", bufs=2, space="PSUM"))
        psz3 = ctx.enter_context(tc.tile_pool(name="psz3", bufs=3, space="PSUM"))
        psct = ctx.enter_context(tc.tile_pool(name="psct", bufs=1, space="PSUM"))

        rt_tiles = {}

        def fetch(s, wide=False):
            # flat [128, 4104] row tile: cols [a(8),t(2),j(256)] in 0:4096 plus
            # an 8-elem zero pad so even-width (j=256) views can overrun by one.
            # split li 0-1 first so ci=0 compute can start before the full tile
            # lands; multiple queues so the chunks ride different DMA engines.
            rtb = rows.tile([128, 4104], BF16, tag="rtb", name=f"rtb{s}")

            def rv(lo, hi, plo=0, phi=128):
                return rtb[plo:phi, lo * 512 : hi * 512]
            if wide:
                nc.sync.dma_start(out=rv(0, 2, 0, 64), in_=img4b[s, 0:64, 0:2, :, :])
                nc.scalar.dma_start(out=rv(0, 2, 64, 128), in_=img4b[s, 64:128, 0:2, :, :])
                nc.sync.dma_start(out=rv(2, 5), in_=img4b[s, :, 2:5, :, :])
                nc.scalar.dma_start(out=rv(5, 8), in_=img4b[s, :, 5:8, :, :])
            else:
                nc.sync.dma_start(out=rv(0, 2), in_=img4b[s, :, 0:2, :, :])
                nc.scalar.dma_start(out=rv(2, 8), in_=img4b[s, :, 2:8, :, :])
            nc.vector.memset(rtb[:, 4096:4104], 0.0)
            rt_tiles[s] = rtb

        # ---- constants (first fetch launched ahead; l1w/b1 next on the
        # sync queue; the rest spread over idle engine queues) ----
        l1w_t = consts.tile([128, 4, 128], BF16)
        nc.gpsimd.dma_start(out=l1w_t, in_=l1w[:, :, :])
        fetch(0, wide=True)
        b1_t = consts.tile([128, 1], F32)
        nc.sync.dma_start(out=b1_t, in_=b1v[:, :])
        l2w_t = consts.tile([128, 128], BF16)
        nc.gpsimd.dma_start(out=l2w_t, in_=l2w[:, :])
        b2_t = consts.tile([128, 1], F32)
        nc.gpsimd.dma_start(out=b2_t, in_=b2v[:, :])
        l3w_t = consts.tile([128, 4, 128], BF16)
        nc.scalar.dma_start(out=l3w_t, in_=l3w[:, :, :])
        b3_t = consts.tile([128, 4], F32)
        nc.scalar.dma_start(out=b3_t, in_=b3v[:, :])
        cw_t = consts.tile([128, 4, 128], BF16)
        nc.gpsimd.dma_start(out=cw_t, in_=cw4[:, :, :])
        fw_t = consts.tile([128, 4, 128], BF16)
        nc.gpsimd.dma_start(out=fw_t, in_=fw[:, :, :])
        eps_t = consts.tile([128, 1], F32)
        nc.vector.memset(eps_t, 1e-12)
        # fp8 copy of the gather weights for DoubleRow |y|^2 reductions:
        # both k-tile planes hold the same 0/1 gather (exact in fp8)
        cw8_t = consts.tile([128, 4, 2, 128], F8)
        for kt in range(2):
            nc.vector.tensor_copy(cw8_t[:, :, kt, :], cw_t[:, :, :])

        def relu(out, z, bias):
            nc.scalar.activation(out, z, mybir.ActivationFunctionType.Relu, bias=bias)

        # per-sample live state for the cross-image pipeline
        ct_banks = {}
        s1_tiles = {}
        sim_tiles = {}
        rf_tiles = {}
        sx_tiles = {}

        def emit_sx(ps):
            # |x| sqrt can issue as soon as the ctx bundle stops (ci2)
            ctx_b = ct_banks[ps][1]
            sx = tailp.tile([128, 512], F32, tag="sx")
            nc.scalar.activation(
                sx, ctx_b[:, :],
                mybir.ActivationFunctionType.Sqrt, bias=eps_t[:, :], scale=16.0,
            )
            sx_tiles[ps] = sx

        def ct_d(ps, ci, proda):
            ctd_b = ct_banks[ps][0]
            for l in range(4):
                nc.tensor.matmul(
                    ctd_b[:, :], cw_t[:, ci, :], proda[:, l % 2, l // 2, :, :],
                    start=(ci == 0 and l == 0), stop=(ci == 3 and l == 3),
                )

        def ct_x(ps, ci):
            # |x|^2: k-sum via cw, l-sum via 4 shifted views of sq (no DVE
            # box filter); depends only on sq so the PE can run it early.
            ctx_b = ct_banks[ps][1]
            sqt = s1_tiles[ps]
            for l in range(4):
                sqv = bass.AP(
                    tensor=sqt.tensor,
                    offset=sqt.offset + ci * 1024 + (l % 2) * 256 + (l // 2),
                    ap=[[4104, 128], [512, 2], [1, 256]],
                )
                nc.tensor.matmul(
                    ctx_b[:, :], cw_t[:, ci, :], sqv,
                    start=(ci == 0 and l == 0), stop=(ci == 3 and l == 3),
                )

        def ct_y(ps, ci, ysqa):
            cty_b = ct_banks[ps][2]
            for jo in range(2):
                nc.tensor.matmul(
                    cty_b[:, :], cw8_t[:, ci, :, :], ysqa[:, :, jo, :, :],
                    start=(ci == 0 and jo == 0), stop=(ci == 3 and jo == 1),
                    perf_mode=mybir.MatmulPerfMode.DoubleRow,
                )

        def alloc_ct(ps):
            # out partition q = 4g+ci holds patch rows (2q, 2q+1) -> fold order.
            ct_banks[ps] = (
                psct.tile([128, 512], F32, tag="ctd", name=f"ctd{ps}"),
                psct.tile([128, 512], F32, tag="ctx", name=f"ctx{ps}"),
                psct.tile([128, 512], F32, tag="cty", name=f"cty{ps}"),
            )

        def emit_sim(ps):
            # cosine tail, PSUM readers only: frees all three ct banks fast.
            # rsx/rsy = 1/sqrt(...); the 1/16 scale pre-divides the fold /4.
            ctd_b, ctx_b, cty_b = ct_banks.pop(ps)
            s1_tiles.pop(ps)
            if ps in sx_tiles:
                sx = sx_tiles.pop(ps)
            else:
                sx = tailp.tile([128, 512], F32, tag="sx")
                nc.scalar.activation(
                    sx, ctx_b[:, :],
                    mybir.ActivationFunctionType.Sqrt, bias=eps_t[:, :], scale=16.0,
                )
            sy = tailp.tile([128, 512], F32, tag="sy")
            nc.scalar.activation(
                sy, cty_b[:, :],
                mybir.ActivationFunctionType.Sqrt, bias=eps_t[:, :],
            )
            d_ = tailp.tile([128, 512], F32, tag="d_")
            nc.vector.tensor_copy(d_, ctd_b[:, :])
            sim_tiles[ps] = (sx, sy, d_)

        def emit_fold_a(ps, drain=False):
            # sim = ctd * rsqrt(16|x|^2) * rsqrt(|y|^2), then column fold:
            # R[q,u,v] = S[2q+u,v-1]+S[2q+u,v], edges doubled
            sx, sy, d_ = sim_tiles.pop(ps)
            m_ = tailp.tile([128, 512], F32, tag="m_")
            if drain:
                nc.vector.tensor_tensor(m_, sx, sy, MULT)
            else:
                nc.gpsimd.tensor_tensor(m_, sx, sy, MULT)
            r_ = tailp.tile([128, 512], F32, tag="r_")
            nc.vector.reciprocal_approx_fast(r_, m_)
            simacc = tailp.tile([128, 2, 256], F32, tag="simacc")
            nc.vector.tensor_tensor(
                simacc.rearrange("p a j -> p (a j)"), d_, r_, MULT,
            )
            rf = foldp.tile([128, 2, 256], BF16, tag="rf")
            nc.vector.tensor_tensor(
                rf[:, :, 1:255], simacc[:, :, 0:254], simacc[:, :, 1:255], ADD
            )
            nc.scalar.activation(
                rf[:, :, 0:1], simacc[:, :, 0:1],
                mybir.ActivationFunctionType.Copy, scale=2.0,
            )
            nc.scalar.activation(
                rf[:, :, 255:256], simacc[:, :, 254:255],
                mybir.ActivationFunctionType.Copy, scale=2.0,
            )
            rf_tiles[ps] = rf

        def emit_fold_b(ps, drain=False):
            # fold rows on the PE (no partition-shift DMA):
            #   tfp[q,0,:] = R[2q-1]+R[2q]   (fw0 = superdiag shift, fw1 = I
            #                                 with [0,0]=2 for the R[-1]:=R[0] edge)
            #   tfp[q,1,:] = R[2q]+R[2q+1]   (fw2 = I with [127,127]=2, fw3 = I
            #                                 with [127,127]=0: junk R[255] masked,
            #                                 row 511 folds 2*R[254])
            q_out2 = nc.scalar if drain else nc.sync
            rf = rf_tiles.pop(ps)
            if drain:
                # keep the PE p-state up while the sim chain finishes
                for wi in range(6):
                    zw = psz.tile([128, 510], F32, tag="z", name=f"dwarm{wi}")
                    nc.tensor.matmul(zw, junk[:, 0:128], junk[:, 0:510], start=True, stop=True)
            tfp = psz3.tile([128, 2, 256], F32, tag="z3", name=f"tfp{ps}")
            nc.tensor.matmul(tfp[:, 0, :], fw_t[:, 0, :], rf[:, 1, :], start=True, stop=False)
            nc.tensor.matmul(tfp[:, 0, :], fw_t[:, 1, :], rf[:, 0, :], start=False, stop=True)
            nc.tensor.matmul(tfp[:, 1, :], fw_t[:, 2, :], rf[:, 0, :], start=True, stop=False)
            nc.tensor.matmul(tfp[:, 1, :], fw_t[:, 3, :], rf[:, 1, :], start=False, stop=True)
            # 2x2 upsample straight out of PSUM: duplicate cols on-chip
            tf2 = foldp.tile([128, 2, 512], BF16, tag="tf2")
            tf2r = tf2.rearrange("p u (v cv) -> p u cv v", cv=2)
            nc.scalar.activation(
                tf2r[:, :, 0, :], tfp, mybir.ActivationFunctionType.Copy
            )
            nc.vector.tensor_copy(tf2r[:, :, 1, :], tfp)
            # contiguous DRAM writes: out4[ps, u, ru, p, :] = tf2[:, u, :]
            # (ru duplication via 0-stride read); host un-permutes rows.
            for u in range(2):
                q = q_out2 if u else nc.sync
                q.dma_start(
                    out=bass.AP(
                        tensor=out4.tensor,
                        offset=out4.offset + (ps * 4 + u * 2) * 128 * IMG,
                        ap=[[IMG, 128], [128 * IMG, 2], [1, IMG]],
                    ),
                    in_=bass.AP(
                        tensor=tf2.tensor,
                        offset=tf2.offset + u * 512,
                        ap=[[1024, 128], [0, 2], [1, 512]],
                    ),
                )

        # ---- PE p-state warmup: junk matmuls under the startup DMA wait ----
        junk = consts.tile([128, 512], BF16)
        nc.vector.memset(junk, 0.0)
        for wi in range(5):
            zw = psz3.tile([128, 512], F32, tag="z3", name=f"warm{wi}")
            nc.tensor.matmul(zw, junk[:, 0:128], junk, start=True, stop=True)

        # Software pipeline: ci's ct matmuls are emitted TWO ci slots later
        # (crossing image boundaries) so the relu->products chain never stalls
        # the PE; the previous image's tail rides along at ci 2-3.
        pendq = []
        for s in range(NSAMP):
            rtb = rt_tiles.pop(s)

            def xv(ci, l):
                # [128, 2(li), 256(j)] view: patch cols j at image col 2j+l
                # (col 255 overruns into the next li row / pad: junk, never read)
                return bass.AP(
                    tensor=rtb.tensor,
                    offset=rtb.offset + ci * 1024 + (l % 2) * 256 + (l // 2),
                    ap=[[4104, 128], [512, 2], [1, 256]],
                )

            for ci in range(4):
                if ci == 2 and s > 0:
                    # previous image's PSUM readers, then reclaim its banks
                    emit_sim(s - 1)
                    alloc_ct(s)
                # ---- layer 1 ----
                z1 = psz.tile([128, 512], F32, tag="z", name=f"z1_{s}_{ci}")
                for l in range(4):
                    nc.tensor.matmul(
                        z1, l1w_t[:, l, :], xv(ci, l), start=(l == 0), stop=(l == 3)
                    )
                h1 = mlp.tile([128, 512], BF16, tag="h1", name=f"h1_{s}_{ci}")
                relu(h1, z1, b1_t[:, :])
                # ---- layer 2 ----
                z2 = psz.tile([128, 512], F32, tag="z", name=f"z2_{s}_{ci}")
                nc.tensor.matmul(z2, l2w_t[:, :], h1, start=True, stop=True)
                h2 = mlp.tile([128, 512], BF16, tag="h2", name=f"h2_{s}_{ci}")
                relu(h2, z2, b2_t[:, :])
                # ---- layer 3 + products ----
                yva = mlp.tile([128, 2, 2, 2, 256], BF16, tag="yva", name=f"yva_{s}_{ci}")
                for l in range(4):
                    z3 = psz3.tile([128, 512], F32, tag="z3", name=f"z3_{s}_{ci}_{l}")
                    nc.tensor.matmul(z3, l3w_t[:, l, :], h2, start=True, stop=True)
                    relu(
                        yva[:, l % 2, l // 2, :, :].rearrange("p a j -> p (a j)"),
                        z3, b3_t[:, l : l + 1],
                    )
                yva_f = yva.rearrange("p t a b j -> p (t a b j)")
                ysqa = mlp.tile([128, 2, 2, 2, 256], F8, tag="ysqa", name=f"ysqa_{s}_{ci}")
                ysqa_f = ysqa.rearrange("p t a b j -> p (t a b j)")
                nc.vector.tensor_tensor(ysqa_f, yva_f, yva_f, MULT)
                # products: one op per jo (clean non-overlapping APs)
                proda = mlp.tile([128, 2, 2, 2, 256], BF16, tag="proda", name=f"proda_{s}_{ci}")
                for jo in range(2):
                    xview = bass.AP(
                        tensor=rtb.tensor,
                        offset=rtb.offset + ci * 1024 + jo,
                        ap=[[4104, 128], [256, 2], [512, 2], [1, 256]],
                    )
                    eng = nc.gpsimd if (jo == 1 and PROD_GPS[ci] and s < NSAMP - 1) else nc.vector
                    eng.tensor_tensor(
                        proda[:, :, jo, :, :], xview, yva[:, :, jo, :, :], MULT
                    )
                pendq.append((s, ci, proda, ysqa))
                if ci == 2 and s > 0:
                    # whole-image |x|^2 bundle: 16 dependency-free matmuls
                    # right where the PE waits on products
                    for cj in range(4):
                        ct_x(s, cj)
                if len(pendq) > 2:
                    p0 = pendq.pop(0)
                    ct_d(*p0[:3])
                    ct_y(p0[0], p0[1], p0[3])

                # ---- interleaved per-ci extras ----
                if ci == 0 and s > 1:
                    emit_fold_b(s - 2)
                if ci == 1:
                    sq = xsq.tile([128, 4104], BF16, tag="sq", name=f"sq{s}")
                    nc.vector.tensor_tensor(
                        sq[:, 0:4096], rtb[:, 0:4096], rtb[:, 0:4096], MULT
                    )
                    nc.vector.memset(sq[:, 4096:4104], 0.0)
                    s1_tiles[s] = sq
                    if s == 0:
                        # first image: banks are fresh, bundle fills the
                        # pipeline-fill gap one ci earlier
                        alloc_ct(0)
                        for cj in range(4):
                            ct_x(0, cj)
                if ci == 3 and s > 0:
                    emit_fold_a(s - 1)
                if ci == 3 and s == NSAMP - 1:
                    p0 = pendq.pop(0)
                    ct_d(*p0[:3])
                    ct_y(p0[0], p0[1], p0[3])
                    emit_sx(s)

            # prefetch next image's row tiles
            if s + 1 < NSAMP:
                fetch(s + 1)

        # drain: last two pending ci groups, then the remaining tails
        emit_fold_b(NSAMP - 2)
        for p0 in pendq:
            ct_d(*p0[:3])
            ct_y(p0[0], p0[1], p0[3])
        emit_sim(NSAMP - 1)
        emit_fold_a(NSAMP - 1, drain=True)
        emit_fold_b(NSAMP - 1, drain=True)

    nc.finalize()
    return nc


def make_weight_inputs(W1, b1, W2, b2, W3, b3):
    """Host-side block-diagonal weight construction."""
    W1 = np.asarray(W1, np.float32)
    W2 = np.asarray(W2, np.float32)
    W3 = np.asarray(W3, np.float32)
    b1 = np.asarray(b1, np.float32)
    b2 = np.asarray(b2, np.float32)
    b3 = np.asarray(b3, np.float32)
    # partition orders: image/z3 rows p = 32k+g ; h1/h2 rows q = 32c+g
    l1w = np.zeros((128, 4, 128), np.float32)
    l2w = np.zeros((128, 128), np.float32)
    l3w = np.zeros((128, 4, 128), np.float32)
    b3v = np.zeros((128, 4), np.float32)
    cwm = np.zeros((128, 4, 128), np.float32)
    for g in range(32):
        for l in range(4):
            for k in range(4):
                for c in range(4):
                    l1w[32 * k + g, l, 32 * c + g] = W1[4 * k + l, c]
                    l3w[32 * c + g, l, 32 * k + g] = W3[c, 4 * k + l]
                b3v[32 * k + g, l] = b3[4 * k + l]
        for k in range(4):
            for ci in range(4):
                cwm[32 * k + g, ci, 4 * g + ci] = 1.0
        for c in range(4):
            for c2 in range(4):
                l2w[32 * c + g, 32 * c2 + g] = W2[c, c2]
    b1v = np.repeat(b1, 32).reshape(128, 1).astype(np.float32)
    b2v = np.repeat(b2, 32).reshape(128, 1).astype(np.float32)
    # row-fold matmul weights: tfp[q,0]=R[2q-1]+R[2q], tfp[q,1]=R[2q]+R[2q+1]
    fw = np.zeros((128, 4, 128), np.float32)
    for q in range(1, 128):
        fw[q - 1, 0, q] = 1.0          # superdiag shift of rf[:,1]
    eye = np.arange(128)
    fw[eye, 1, eye] = 1.0
    fw[0, 1, 0] = 2.0                  # R[-1] := R[0]
    fw[eye, 2, eye] = 1.0
    fw[127, 2, 127] = 2.0              # row 511 folds 2*R[254]
    fw[eye, 3, eye] = 1.0
    fw[127, 3, 127] = 0.0              # mask junk R[255]
    import ml_dtypes

    bf = ml_dtypes.bfloat16
    return {
        "l1w": l1w.astype(bf), "l2w": l2w.astype(bf), "l3w": l3w.astype(bf),
        "b3v": b3v, "cw4": cwm.astype(bf), "fw": fw.astype(bf),
        "b1v": b1v, "b2v": b2v,
    }


_NC = None


def get_nc():
    global _NC
    if _NC is None:
        _NC = build_nc()
    return _NC


def _bf16():
    import ml_dtypes

    return ml_dtypes.bfloat16


def gather_rows(img_n):
    """(n,512,512) f32 -> (n,128,8,2,256) bf16: [p=32k+g, li, t, j] = img[16g+k+2li, 2j+t]."""
    n = img_n.shape[0]
    pad = np.zeros((n, IMG + 4, IMG), np.float32)
    pad[:, :IMG, :] = img_n
    p = np.arange(128)
    li = np.arange(8)
    rows_idx = 16 * (p[:, None] % 32) + (p[:, None] // 32) + 2 * li[None, :]
    out = pad[:, rows_idx, :]                       # (n,128,8,512)
    out = out.reshape(n, 128, 8, 256, 2).transpose(0, 1, 2, 4, 3)  # (n,128,8,2,256)
    return np.ascontiguousarray(out).astype(_bf16())


def make_in_maps(img, wts):
    in_maps = []
    for c in range(NCORES):
        chunk = img[c * NSAMP : (c + 1) * NSAMP]
        m = {"img4b": gather_rows(chunk)}
        m.update(wts)
        in_maps.append(m)
    return in_maps


def kernel(img, W1, b1, W2, b2, W3, b3):
    from concourse.bass_utils import run_bass_kernel_spmd

    img = np.asarray(img, np.float32).reshape(32, IMG, IMG)
    wts = make_weight_inputs(W1, b1, W2, b2, W3, b3)
    nc = get_nc()
    in_maps = make_in_maps(img, wts)
    res = run_bass_kernel_spmd(nc, in_maps, list(range(NCORES)))
    parts = []
    for i in range(NCORES):
        a = np.asarray(res.results[i]["out4"])  # [4, 2, 2, 128, 512] (u, ru, p, c)
        parts.append(a.transpose(0, 3, 1, 2, 4).reshape(NSAMP, IMG, IMG))
    return np.concatenate(parts, axis=0).astype(np.float32)
